# revision 56
# baseline (speedup 1.0000x reference)
"""Adaptive-attention LSTM (B=32,T=64,HID=512,K=L=49,VOCAB=10000) on 8 TRN2 cores.

Strategy
--------
Data-parallel over batch: each core gets B_local=4 sequences. Everything is
computed in a "transposed" layout [feature-on-partition, (t,b)-on-free] so the
serial LSTM cell needs no transposes:

  phase A (batched): x_gates_T = Wih@x + (bih+bhh), x_sent_T = Wx_s@x,
                     Vk = V@Wv.T (replicated to [128, L*K] via a DRAM bounce)
  phase B (serial, 64 steps): only the LSTM cell. gates_T[2048, 4] accumulated
                     in PSUM from 64 bf16 LDW+matmuls (Whh.T stationary);
                     sigma/tanh on ACT, c/h updates on DVE. h_t, tanh(c_t)
                     stored for phase C.
  phase C (batched): spatial attention z/alpha, visual sentinel s/beta,
                     c_hat folded as  u = (e_how*s + e_z@V)/D + h  with
                     D = sum(e_z) + e_how  (exact softmax algebra; safe without
                     max-subtraction since |z| < 1), then the dominant GEMM
                     y = u @ Wmlp.T + bmlp with Wmlp streamed from HBM.

Host side only shards/transposes/casts inputs (bf16 for matmul operands) and
reassembles the output.
"""

import os
import sys
import types

for _p in ("/opt/pypackages", "/opt/trn_rl_repo"):
    if _p not in sys.path and os.path.isdir(_p):
        sys.path.insert(0, _p)


def _install_ntff_shim():
    """antenv.axon_hooks is missing in the trimmed repo; provide it so
    run_bass_kernel_spmd(trace=True) can reach the NTFF profile hook."""
    if "antenv.axon_hooks" in sys.modules:
        return
    try:
        from trn_agent_boot.trn_boot import _ntff_profile_via_ctypes

        hook = _ntff_profile_via_ctypes("/opt/axon/libaxon_pjrt.so")
    except Exception:
        hook = None
    m = types.ModuleType("antenv.axon_hooks")
    m.get_axon_ntff_profile_hook = lambda: hook
    m.set_axon_ntff_profile_hook = lambda h: None
    sys.modules["antenv.axon_hooks"] = m


_install_ntff_shim()


def _enable_ldw_opt():
    """compile flags hardcode --enable-ldw-opt=false; flip it so LDWEIGHTS can
    use the fast-weight-load path (bf16 weight tiles load 2x faster)."""
    import concourse.bass_utils as bu

    if os.environ.get("BASS_LDW_OPT", "1") != "1":
        return
    if getattr(bu, "_ldw_patched", False):
        return
    orig = bu.run_command

    def patched(argv, **kw):
        argv = [a.replace("--enable-ldw-opt=false", "--enable-ldw-opt=true")
                if isinstance(a, str) else a for a in argv]
        return orig(argv, **kw)

    bu.run_command = patched
    bu._ldw_patched = True


_enable_ldw_opt()

import ml_dtypes
import numpy as np

import concourse.bass as bass
import concourse.mybir as mybir
import concourse.tile as tile
from concourse.bass_utils import run_bass_kernel_spmd
from concourse.tile import add_dep_helper
from concourse.vector_clock import ScopedClock


def _patch_tile_drain():
    """This walrus build allows a single sync-wait per CTRL instruction; Tile's
    tail drain attaches one wait per live semaphore. Spread them over a chain
    of SP nops (sequential on SP -> identical semantics)."""
    if getattr(tile.TileContext, "_drain_patched", False):
        return

    def _drain_and_barrier(self, tick_clock, wait_clock):
        nc = self.nc
        probe = nc.sync.nop(nofuse=True)
        wait_clock.add_sem_waits(
            probe.ins, ScopedClock({None: tick_clock.global_clock})
        )
        waits = list(probe.ins.sync_info.on_wait)
        if len(waits) > 1:
            probe.ins.sync_info.on_wait = waits[:1]
            for i in range(1, len(waits)):
                extra = nc.sync.nop(nofuse=True)
                if extra.ins.sync_info is None:
                    extra.ins.sync_info = mybir.SyncInfo(
                        on_wait=waits[i : i + 1], on_update=[]
                    )
                else:
                    extra.ins.sync_info.on_wait = waits[i : i + 1]
        nc.sync.drain()
        nc.all_engine_barrier()
        assert self.sems is not None
        popped = nc._tile_sem_poison_stack.pop()
        assert popped is self._sem_poison
        nc.clear_and_free_semaphores(list(self.sems.allocated().values()))
        nc.all_engine_barrier()

    tile.TileContext._drain_and_barrier = _drain_and_barrier
    tile.TileContext._drain_patched = True

    # General pass: the ISA here allows a single sync-wait per instruction.
    # Before lowering, split any instruction with N>1 waits into N-1 preceding
    # single-wait NOPs on the same engine (engine streams execute in order, so
    # semantics are identical).
    _orig_lower = tile.TileContext._lower_ordered_insts

    def _split_multi_waits(self, ordered):
        nc = self.nc
        # Sweep 1: fuse standalone InstLdweights back into their InstMatmult
        # (self-loading form, ldweights=True). The standalone form defeats the
        # codegen fast-weight-load path (LDWEIGHTS measured at the slow
        # 128col/1.2GHz rate). Pair FIFO by PE stream order; merge sync lists.
        fuse = os.environ.get("BASS_FUSE_LDW", "1") == "1"
        for insts in ordered.values():
            if not fuse:
                break
            pending = []
            fused_out = []
            for inst in insts:
                tn = type(inst).__name__
                if tn == "InstLdweights":
                    pending.append(inst)
                    continue
                if tn == "InstMatmult" and getattr(inst, "ldweights", None) is not True \
                        and pending:
                    want = str(inst.ins[1])
                    idx = next((j for j, l in enumerate(pending)
                                if str(l.ins[0]) == want), 0)
                    ldw = pending.pop(idx)
                    inst.ldweights = True
                    lsi = getattr(ldw, "sync_info", None)
                    if lsi is not None and (lsi.on_wait or lsi.on_update):
                        si = inst.sync_info
                        if si is None:
                            inst.sync_info = mybir.SyncInfo(
                                on_wait=list(lsi.on_wait or []),
                                on_update=list(lsi.on_update or []),
                            )
                        else:
                            si.on_wait = list(lsi.on_wait or []) + list(si.on_wait or [])
                            si.on_update = list(si.on_update or []) + list(
                                lsi.on_update or []
                            )
                fused_out.append(inst)
            assert not pending, f"{len(pending)} unpaired Ldweights"
            insts[:] = fused_out

        # Sweep 2: this ISA allows one sync-wait per instruction; spill extras
        # onto preceding same-engine NOPs.
        for insts in ordered.values():
            out = []
            for inst in insts:
                si = getattr(inst, "sync_info", None)
                eng = getattr(inst, "engine", None)
                if si is not None and eng is not None and si.on_wait is not None \
                        and len(si.on_wait) > 1:
                    waits = list(si.on_wait)
                    for w in waits[:-1]:
                        out.append(mybir.InstNoOp(
                            name=nc.get_next_instruction_name(),
                            engine=eng,
                            bass_nofuse=True,
                            sync_info=mybir.SyncInfo(on_wait=[w], on_update=[]),
                        ))
                    si.on_wait = waits[-1:]
                out.append(inst)
            insts[:] = out
        return _orig_lower(self, ordered)

    tile.TileContext._lower_ordered_insts = _split_multi_waits


_patch_tile_drain()

F32 = mybir.dt.float32
BF16 = mybir.dt.bfloat16
FP8 = mybir.dt.float8e4
BF = ml_dtypes.bfloat16
F8 = ml_dtypes.float8_e4m3fn
AF = mybir.ActivationFunctionType
ALU = mybir.AluOpType

HID = 512
INP = 512
K = 49
L = 49
VOCAB = 10000
B, T = 32, 64
NCORES = 8
BL = B // NCORES          # 4 sequences per core
NBT = BL * T              # 256 (t-major: col = t*BL + b)
G4 = 4 * HID              # 2048
NKC = HID // 128          # 4 k-chunks
NMT = G4 // 128           # 16 gate m-tiles
NCH = 20                  # vocab chunks
CHN = VOCAB // NCH        # 500

LAST_RESULT = None        # BassKernelResults of the most recent run (for test.py)


def _bcast_ap(ap2d, count, pos=1):
    """Insert a zero-stride dim of `count` at free position `pos` of a 2-D AP."""
    dims = list(ap2d.ap)
    dims.insert(pos, [0, count])
    return bass.AP(tensor=ap2d.tensor, offset=ap2d.offset, ap=dims)


def _part_bcast(dram_ap, parts):
    """DRAM AP replicated over `parts` partitions (zero-stride partition dim)."""
    return bass.AP(
        tensor=dram_ap.tensor, offset=dram_ap.offset,
        ap=[[0, parts]] + list(dram_ap.ap),
    )


def _build():
    nc = bass.Bass()

    d_xT = nc.dram_tensor("xT", [INP, NBT], BF16, kind="ExternalInput")
    d_vT = nc.dram_tensor("vT", [HID, BL * L], BF16, kind="ExternalInput")
    d_vnat = nc.dram_tensor("vnat", [BL * L, HID], BF16, kind="ExternalInput")
    d_wihT = nc.dram_tensor("wihT", [INP, G4], BF16, kind="ExternalInput")
    d_whhT = nc.dram_tensor("whhT", [HID, G4], BF16, kind="ExternalInput")
    d_wxsT = nc.dram_tensor("wxsT", [INP, HID], BF16, kind="ExternalInput")
    d_whsT = nc.dram_tensor("whsT", [HID, HID], BF16, kind="ExternalInput")
    d_wgT = nc.dram_tensor("wgT", [HID, K], BF16, kind="ExternalInput")
    d_wsT = nc.dram_tensor("wsT", [HID, K], BF16, kind="ExternalInput")
    d_wvT = nc.dram_tensor("wvT", [HID, K], BF16, kind="ExternalInput")
    d_wmlpT = nc.dram_tensor("wmlpT", [HID, VOCAB], BF16, kind="ExternalInput")
    d_b4 = nc.dram_tensor("b4", [128, NMT], F32, kind="ExternalInput")
    d_wh = nc.dram_tensor("wh", [K], BF16, kind="ExternalInput")
    d_bmlp = nc.dram_tensor("bmlp", [VOCAB], BF16, kind="ExternalInput")
    d_y = nc.dram_tensor("y", [NBT, VOCAB], BF16, kind="ExternalOutput")

    ident = nc.inline_tensor(np.eye(128, dtype=np.float32), name="ident128")
    ident_bf = nc.inline_tensor(
        np.eye(128, dtype=np.float32).astype(ml_dtypes.bfloat16), name="ident128bf"
    )

    from contextlib import ExitStack

    with tile.TileContext(nc) as tc, ExitStack() as es:
        consts = es.enter_context(tc.tile_pool(name="consts", bufs=1))
        state = es.enter_context(tc.tile_pool(name="state", bufs=1))
        dram = es.enter_context(tc.tile_pool(name="dram", bufs=1, space="DRAM"))

        # ---- constant / weight loads ------------------------------------
        def load_kc(drm, cols, name, dt=BF16, eng=None):
            # single strided DMA: DRAM [(kc p), cols] -> SBUF [p, kc, cols]
            # (one queue issue instead of NKC; the issue cost ~650ns dominates)
            t = consts.tile([128, NKC, cols], dt, tag=name)
            src = drm[:]
            (eng or nc.sync).dma_start(
                out=t[:],
                in_=bass.AP(tensor=src.tensor, offset=src.offset,
                            ap=[[cols, 128], [128 * cols, NKC], [1, cols]]),
            )
            return t

        # Queue split: the ACT queue (scalar) gets only small/late tensors and
        # issues them FIRST -- its DMA issues block the ACT instruction stream,
        # so nothing big may sit in front of the phase-A psum->xg copies. The
        # big x-path weights stream on the SP queue in dependency order.
        b4 = consts.tile([128, NMT], F32, tag="b4")
        nc.scalar.dma_start(out=b4[:], in_=d_b4[:])
        vT = load_kc(d_vT, BL * L, "vT", eng=nc.scalar)
        wvT = load_kc(d_wvT, K, "wvT", eng=nc.scalar)
        wgT = load_kc(d_wgT, K, "wgT", eng=nc.scalar)
        wsT = load_kc(d_wsT, K, "wsT", eng=nc.scalar)
        whsT = load_kc(d_whsT, HID, "whsT", eng=nc.scalar)
        vnat = consts.tile([L, BL, HID], BF16, tag="vnat")
        vsrc = d_vnat[:]
        nc.scalar.dma_start(
            out=vnat[:],
            in_=bass.AP(tensor=vsrc.tensor, offset=vsrc.offset,
                        ap=[[HID, L], [L * HID, BL], [1, HID]]),
        )

        xT = load_kc(d_xT, NBT, "xT")
        wihT = load_kc(d_wihT, G4, "wihT")
        wxsT = load_kc(d_wxsT, HID, "wxsT")
        whhT = load_kc(d_whhT, G4, "whhT")

        ones_2b = consts.tile([128, 2, BL], F32, tag="ones_2b")
        nc.vector.memset(ones_2b[:], 1.0)
        wh_rep = consts.tile([128, K], BF16, tag="wh_rep")
        nc.scalar.dma_start(out=wh_rep[:], in_=_part_bcast(d_wh[:], 128))
        bmlp_rep = consts.tile([128, VOCAB], BF16, tag="bmlp_rep")
        # 2.5MB broadcast only needed by the MLP tail: keep it on the idle
        # SWDGE queue so it blocks neither HWDGE queue
        nc.gpsimd.dma_start(out=bmlp_rep[:], in_=_part_bcast(d_bmlp[:], 128))
        id_sb = consts.tile([128, 128], F32, tag="ident")
        nc.scalar.dma_start(out=id_sb[:], in_=ident[:])
        id_bf = consts.tile([128, 128], BF16, tag="ident_bf")
        nc.scalar.dma_start(out=id_bf[:], in_=ident_bf[:])
        ones1 = consts.tile([1, 128], F32, tag="ones1")
        nc.vector.memset(ones1[:], 1.0)

        # ---- persistent state -------------------------------------------
        xg = state.tile([128, T, NMT, BL], BF16, tag="xg")
        xs = state.tile([128, NKC, T, BL], F32, tag="xs")
        tanhc_bf = state.tile([128, NKC, T, BL], BF16, tag="tanhc_bf")
        h_bf = state.tile([128, NKC, T + 1, BL], BF16, tag="h_bf")
        c_st = state.tile([128, NKC, BL], F32, tag="c_st")
        vk_rep = state.tile([128, L, K], BF16, tag="vk_rep")
        s_bf = state.tile([128, NKC, NBT], BF16, tag="s_bf")
        u_bf = state.tile([128, NKC, NBT], BF16, tag="u_bf")
        hg_sb = state.tile([128, 2, K], BF16, tag="hg_sb")
        ez_sb = state.tile([128, 2, K], F32, tag="ez_sb")
        zt_sb = state.tile([128, 2, K], F32, tag="zt_sb")
        sws_sb = state.tile([128, 2, K], F32, tag="sws_sb")
        ezs_f = state.tile([128, 2, K], F32, tag="ezs_f")
        ezT = state.tile([L, NBT], BF16, tag="ezT")
        scal = state.tile([128, 2, 8], F32, tag="scal")
        frow = state.tile([1, NBT], F32, tag="frow")
        fbc = state.tile([128, NBT], F32, tag="fbc")

        d_vk = dram.tile([BL * L, K], BF16, tag="d_vk")
        d_vkrep = dram.tile([128, L * K], BF16, tag="d_vkrep")

        wmlp_sb = state.tile([128, NKC, VOCAB], BF16, tag="wmlp_sb")

        nc.vector.memset(c_st[:], 0.0)
        nc.vector.memset(h_bf[:, :, 0, :], 0.0)

        # ================= phase A: batched input projections =============
        # all 16 xg psum groups stay open (kc-outer) so the matmul stream is
        # never paced by the psum->xg ACT copies; copies drain at the end
        with tc.tile_pool(name="pa_psum", bufs=8, space="PSUM") as pa_psum:
            prev_mm = None
            for wave in range(2):
                pgs = [pa_psum.tile([128, NBT], F32, tag="pa", name=f"pg{wave}_{i}")
                       for i in range(8)]
                for kc in range(NKC):
                    for i in range(8):
                        mt = wave * 8 + i
                        mm = nc.tensor.matmul(
                            pgs[i][:],
                            lhsT=wihT[:, kc, mt * 128 : (mt + 1) * 128],
                            rhs=xT[:, kc, :],
                            start=(kc == 0),
                            stop=(kc == NKC - 1),
                            skip_group_check=True,
                        )
                        if prev_mm is not None:
                            add_dep_helper(mm.ins, prev_mm.ins, sync=False,
                                           reason="psum group order")
                        prev_mm = mm
                for i in range(8):
                    mt = wave * 8 + i
                    nc.scalar.activation(
                        out=xg[:, :, mt, :],
                        in_=pgs[i][:].rearrange("p (t b) -> p t b", b=BL),
                        func=AF.Identity,
                        bias=b4[:, mt : mt + 1],
                        scale=1.0,
                    )

        # ================= phase B: serial LSTM recurrence ================
        # Layout: gates permuted chunk-major on host: 2 chunks of hidden dims
        # (0:256, 256:512); within a chunk the 8 m-tiles are [i,i,f,f,o,o,g,g]
        # and the g rows are pre-scaled by 2 so tanh(g) = 2*sigmoid(2g)-1 and
        # ONE sigmoid covers all 32 psum cols. x_gates is DMA-preloaded into
        # PSUM (matmuls accumulate with start=False), so the chain per chunk is
        # sigmoid -> [stt tanh_g, cf, ig, c+] -> tanh_c -> h, with chunk 0's
        # chain overlapping chunk 1's matmuls and the next step's matmuls.
        with tc.tile_pool(name="pb_psum", bufs=4, space="PSUM") as pb_psum, \
             tc.tile_pool(name="pb_tmp", bufs=4) as pb_tmp:
            # stream the full Wmlp into SBUF on the otherwise-idle DMA queue
            # while the recurrence runs; the MLP phase then never touches HBM
            wsrc = d_wmlpT[:]
            nc.sync.dma_start(
                out=wmlp_sb[:],
                in_=bass.AP(tensor=wsrc.tensor, offset=wsrc.offset,
                            ap=[[VOCAB, 128], [128 * VOCAB, NKC], [1, VOCAB]]),
            )
            for t in range(T):
                pcs = []
                prev_mm = None
                for c in range(2):
                    pc = pb_psum.tile([128, 8, BL], F32, tag="pg")
                    # seed psum with x_gates via an identity matmul on the
                    # half-idle PE (keeps the preload off the busy ACT queue)
                    mm = nc.tensor.matmul(
                        pc[:].rearrange("p a b -> p (a b)"),
                        lhsT=id_bf[:],
                        rhs=xg[:, t, c * 8 : (c + 1) * 8, :].rearrange(
                            "p a b -> p (a b)"),
                        start=True, stop=False, skip_group_check=True,
                    )
                    if prev_mm is not None:
                        add_dep_helper(mm.ins, prev_mm.ins, sync=False,
                                       reason="psum group order")
                    prev_mm = mm
                    pcs.append(pc)
                for c in range(2):
                    for ic in range(2):
                        for ml in range(8):
                            mt = c * 8 + ml
                            for kk in range(2):
                                kc = ic * 2 + kk
                                mm = nc.tensor.matmul(
                                    pcs[c][:, ml, :],
                                    lhsT=whhT[:, kc, mt * 128 : (mt + 1) * 128],
                                    rhs=h_bf[:, kc, t, :],
                                    start=False,
                                    stop=(ic == 1 and kk == 1),
                                    skip_group_check=True,
                                )
                                if prev_mm is not None:
                                    add_dep_helper(mm.ins, prev_mm.ins, sync=False,
                                                   reason="psum group order")
                                prev_mm = mm

                acts = []
                for c in range(2):
                    a = pb_tmp.tile([128, 8, BL], F32, tag="act")
                    nc.scalar.activation(out=a[:], in_=pcs[c][:], func=AF.Sigmoid)
                    acts.append(a)
                for c in range(2):
                    a = acts[c]
                    cs = c_st[:, 2 * c : 2 * c + 2, :]
                    nc.vector.scalar_tensor_tensor(
                        out=a[:, 6:8, :], in0=a[:, 6:8, :], scalar=2.0,
                        in1=ones_2b[:], op0=ALU.mult, op1=ALU.subtract,
                    )
                    nc.vector.tensor_mul(cs, a[:, 2:4, :], cs)
                    ig = pb_tmp.tile([128, 2, BL], F32, tag="ig")
                    nc.vector.tensor_mul(ig[:], a[:, 0:2, :], a[:, 6:8, :])
                    nc.vector.tensor_add(cs, cs, ig[:])
                    nc.scalar.activation(
                        out=tanhc_bf[:, 2 * c : 2 * c + 2, t, :], in_=cs, func=AF.Tanh
                    )
                for c in range(2):
                    nc.vector.tensor_mul(
                        h_bf[:, 2 * c : 2 * c + 2, t + 1, :],
                        acts[c][:, 4:6, :],
                        tanhc_bf[:, 2 * c : 2 * c + 2, t, :],
                    )

        with tc.tile_pool(name="pa2_psum_late", bufs=4, space="PSUM") as pa_psum:
            pss = [pa_psum.tile([128, NBT], F32, tag="pa", name=f"ps{st}")
                   for st in range(NKC)]
            prev_mm = None
            for kc in range(NKC):
                for st in range(NKC):
                    mm = nc.tensor.matmul(
                        pss[st][:],
                        lhsT=wxsT[:, kc, st * 128 : (st + 1) * 128],
                        rhs=xT[:, kc, :],
                        start=(kc == 0),
                        stop=(kc == NKC - 1),
                        skip_group_check=True,
                    )
                    if prev_mm is not None:
                        add_dep_helper(mm.ins, prev_mm.ins, sync=False,
                                       reason="psum group order")
                    prev_mm = mm
            for st in range(NKC):
                nc.scalar.copy(
                    out=xs[:, st, :, :],
                    in_=pss[st][:].rearrange("p (t b) -> p t b", b=BL),
                )

            # Vk = V @ Wv.T  ->  DRAM bounce  ->  [128, L*K] partition replica
            for half in range(2):
                rows = 128 if half == 0 else BL * L - 128
                p = pa_psum.tile([128, K], F32, tag="pvk")
                for kc in range(NKC):
                    nc.tensor.matmul(
                        p[:rows, :],
                        lhsT=vT[:, kc, half * 128 : half * 128 + rows],
                        rhs=wvT[:, kc, :],
                        start=(kc == 0),
                        stop=(kc == NKC - 1),
                    )
                tmp = state.tile([128, K], BF16, tag=f"vkh{half}")
                nc.scalar.copy(out=tmp[:rows, :], in_=p[:rows, :])
                nc.sync.dma_start(
                    out=d_vk[half * 128 : half * 128 + rows, :], in_=tmp[:rows, :]
                )
            src = d_vk[:]
            dst = d_vkrep[:]
            nc.sync.dma_start(
                out=bass.AP(tensor=dst.tensor, offset=dst.offset,
                            ap=[[L * K * BL, 128 // BL], [L * K, BL], [1, L * K]]),
                in_=bass.AP(tensor=src.tensor, offset=src.offset,
                            ap=[[0, 128 // BL], [L * K, BL], [1, L * K]]),
            )
            nc.sync.dma_start(
                out=vk_rep[:].rearrange("p l k -> p (l k)"), in_=d_vkrep[:]
            )

        # ================= phase C: attention + sentinel ==================
        with tc.tile_pool(name="pc_psum", bufs=3, space="PSUM") as pc_psum, \
             tc.tile_pool(name="pc_cat", bufs=4, space="PSUM") as pc_cat, \
             tc.tile_pool(name="pc_tmp", bufs=2) as pc_tmp:
            # pin ACT order within phase C so the scheduler cannot interleave
            # EXP ops (different function table) with tanh/sigmoid ones: each
            # interleaving costs a 1.3us ACT_TABLE_LOAD
            _act_prev = [None]

            def _ach(op):
                if _act_prev[0] is not None:
                    add_dep_helper(op.ins, _act_prev[0].ins, sync=False,
                                   reason="act table order")
                _act_prev[0] = op
                return op

            for ti in range(2):
                cols = slice(1 + ti * 32, 1 + ti * 32 + 32)   # current h
                pcols = slice(0 + ti * 32, 0 + ti * 32 + 32)  # h_prev (shifted)
                bt = slice(ti * 128, (ti + 1) * 128)

                ph = pc_psum.tile([128, K], F32, tag="pcp")
                for kc in range(NKC):
                    nc.tensor.matmul(
                        ph[:], lhsT=h_bf[:, kc, cols, :], rhs=wgT[:, kc, :],
                        start=(kc == 0), stop=(kc == NKC - 1),
                    )
                _ach(nc.scalar.copy(out=hg_sb[:, ti, :], in_=ph[:]))

                # z-chain in half-L slices: DVE (add/mul/reduce) and ACT (tanh)
                # ping-pong across halves instead of serializing one big chain
                zin = pc_tmp.tile([128, L, K], BF16, tag="zin")
                for l0, l1 in ((0, 25), (25, L)):
                    nc.vector.tensor_add(
                        zin[:, l0:l1, :], vk_rep[:, l0:l1, :],
                        _bcast_ap(hg_sb[:, ti, :], l1 - l0),
                    )
                    _ach(nc.scalar.activation(
                        out=zin[:, l0:l1, :], in_=zin[:, l0:l1, :], func=AF.Tanh
                    ))
                    nc.vector.tensor_mul(
                        zin[:, l0:l1, :], zin[:, l0:l1, :],
                        _bcast_ap(wh_rep[:], l1 - l0),
                    )
                    nc.vector.tensor_reduce(
                        out=zt_sb[:, ti, l0:l1], in_=zin[:, l0:l1, :],
                        axis=mybir.AxisListType.X, op=ALU.add,
                    )

                for st in range(NKC):
                    ps = pc_psum.tile([128, 128], F32, tag="pcp")
                    for kc in range(NKC):
                        nc.tensor.matmul(
                            ps[:], lhsT=whsT[:, kc, st * 128 : (st + 1) * 128],
                            rhs=h_bf[:, kc, pcols, :],
                            start=(kc == 0), stop=(kc == NKC - 1),
                        )
                    pssb = pc_tmp.tile([128, 128], F32, tag="pssb")
                    nc.vector.tensor_add(
                        pssb[:].rearrange("p (t b) -> p t b", b=BL),
                        ps[:].rearrange("p (t b) -> p t b", b=BL),
                        xs[:, st, ti * 32 : (ti + 1) * 32, :],
                    )
                    _ach(nc.scalar.activation(out=pssb[:], in_=pssb[:], func=AF.Sigmoid))
                    nc.vector.tensor_mul(
                        s_bf[:, st, bt].rearrange("p (t b) -> p t b", b=BL),
                        pssb[:].rearrange("p (t b) -> p t b", b=BL),
                        tanhc_bf[:, st, ti * 32 : (ti + 1) * 32, :],
                    )

                pw = pc_psum.tile([128, K], F32, tag="pcp")
                for kc in range(NKC):
                    nc.tensor.matmul(
                        pw[:], lhsT=s_bf[:, kc, bt], rhs=wsT[:, kc, :],
                        start=(kc == 0), stop=(kc == NKC - 1),
                    )
                sws = sws_sb[:, ti, :]
                nc.vector.tensor_add(sws, pw[:], hg_sb[:, ti, :])
                _ach(nc.scalar.activation(out=sws, in_=sws, func=AF.Tanh))
                nc.vector.tensor_mul(sws, sws, wh_rep[:])
                nc.vector.tensor_reduce(
                    out=scal[:, ti, 1:2], in_=sws,
                    axis=mybir.AxisListType.X, op=ALU.add,
                )

            # pass 2: all exp/softmax work together so the ACT function table
            # switches to EXP exactly once instead of per-ti interleaving
            for ti in range(2):
                bt = slice(ti * 128, (ti + 1) * 128)
                _ach(nc.scalar.activation(out=ez_sb[:, ti, :], in_=zt_sb[:, ti, :], func=AF.Exp))
                _ach(nc.scalar.activation(out=scal[:, ti, 1:2], in_=scal[:, ti, 1:2], func=AF.Exp))
                nc.vector.tensor_reduce(
                    out=scal[:, ti, 0:1], in_=ez_sb[:, ti, :],
                    axis=mybir.AxisListType.X, op=ALU.add,
                )
                nc.vector.tensor_add(scal[:, ti, 2:3], scal[:, ti, 0:1], scal[:, ti, 1:2])
                nc.vector.reciprocal(scal[:, ti, 3:4], scal[:, ti, 2:3])
                nc.vector.tensor_mul(scal[:, ti, 4:5], scal[:, ti, 1:2], scal[:, ti, 3:4])
                nc.vector.tensor_scalar_mul(ezs_f[:, ti, :], ez_sb[:, ti, :], scal[:, ti, 3:4])
                pt = pc_psum.tile([K, 128], F32, tag="pcp")
                nc.tensor.transpose(pt[:], ezs_f[:, ti, :], id_sb[:])
                nc.vector.tensor_copy(ezT[:, bt], pt[:])
                pf = pc_psum.tile([1, 128], F32, tag="pcp")
                nc.tensor.transpose(pf[:], scal[:, ti, 4:5], id_sb[:])
                nc.vector.tensor_copy(frow[:, bt], pf[:])

            pfb = pc_psum.tile([128, NBT], F32, tag="pcp")
            nc.tensor.matmul(pfb[:], lhsT=ones1[:], rhs=frow[:], start=True, stop=True)
            nc.vector.tensor_copy(fbc[:], pfb[:])

            catts = []
            for hc in range(NKC):
                pc = pc_cat.tile([128, NBT], F32, tag="catt")
                for b in range(BL):
                    nc.tensor.matmul(
                        pc[:].rearrange("p (t b) -> p t b", b=BL)[:, :, b],
                        lhsT=vnat[:, b, hc * 128 : (hc + 1) * 128],
                        rhs=ezT[:].rearrange("p (t b) -> p t b", b=BL)[:, :, b],
                        start=True, stop=True,
                    )
                catts.append(pc)

            for hc in range(NKC):
                us = pc_tmp.tile([128, NBT], F32, tag="us")
                nc.vector.tensor_mul(us[:], s_bf[:, hc, :], fbc[:])
                nc.vector.tensor_add(us[:], us[:], catts[hc][:])
                nc.vector.tensor_add(
                    us[:].rearrange("p (t b) -> p t b", b=BL),
                    us[:].rearrange("p (t b) -> p t b", b=BL),
                    h_bf[:, hc, 1 : T + 1, :],
                )
                nc.vector.tensor_copy(u_bf[:, hc, :], us[:])

        # ================= MLP: y = u @ Wmlp.T + bmlp =====================
        with tc.tile_pool(name="mlp_ps", bufs=6, space="PSUM") as mlp_ps, \
             tc.tile_pool(name="mlp_out", bufs=6) as mlp_out:
            ydst = d_y[:]
            for nch in range(NCH):
                ysb = mlp_out.tile([128, 2, CHN], BF16, tag="ysb")
                for ti in range(2):
                    py = mlp_ps.tile([128, CHN], F32, tag="py")
                    for kc in range(NKC):
                        nc.tensor.matmul(
                            py[:], lhsT=u_bf[:, kc, ti * 128 : (ti + 1) * 128],
                            rhs=wmlp_sb[:, kc, nch * CHN : (nch + 1) * CHN],
                            start=(kc == 0), stop=(kc == NKC - 1),
                        )
                    nc.vector.tensor_add(
                        ysb[:, ti, :], py[:], bmlp_rep[:, nch * CHN : (nch + 1) * CHN]
                    )
                # one coalesced store per vocab chunk on the idle ACT queue
                nc.scalar.dma_start(
                    out=bass.AP(tensor=ydst.tensor, offset=ydst.offset + nch * CHN,
                                ap=[[VOCAB, 128], [128 * VOCAB, 2], [1, CHN]]),
                    in_=ysb[:],
                )

    return nc


_NC_CACHE = None


def kernel(**inputs):
    global _NC_CACHE, LAST_RESULT
    x = np.asarray(inputs["x"], np.float32)
    V = np.asarray(inputs["V"], np.float32)
    Wih = np.asarray(inputs["Wih"], np.float32)
    Whh = np.asarray(inputs["Whh"], np.float32)
    bih = np.asarray(inputs["bih"], np.float32)
    bhh = np.asarray(inputs["bhh"], np.float32)
    Wx_s = np.asarray(inputs["Wx_s"], np.float32)
    Wh_s = np.asarray(inputs["Wh_s"], np.float32)
    Wv = np.asarray(inputs["Wv"], np.float32)
    Wg = np.asarray(inputs["Wg"], np.float32)
    Wh_att = np.asarray(inputs["Wh_att"], np.float32)
    Ws = np.asarray(inputs["Ws"], np.float32)
    Wmlp = np.asarray(inputs["Wmlp"], np.float32)
    bmlp = np.asarray(inputs["bmlp"], np.float32)

    if _NC_CACHE is None:
        _NC_CACHE = _build()
    nc = _NC_CACHE

    # permute gates chunk-major: chunk c (hidden dims c*256:(c+1)*256) holds
    # [i, f, o, g] blocks of 256; g rows are pre-scaled by 2 so the kernel can
    # compute tanh(g) = 2*sigmoid(2g) - 1 with a single fused sigmoid.
    perm = np.concatenate([
        np.concatenate([
            np.arange(gb * 512 + c * 256, gb * 512 + (c + 1) * 256)
            for gb in (0, 1, 3, 2)  # i, f, o, g
        ])
        for c in (0, 1)
    ])
    gscale = np.ones(4 * HID, np.float32)
    gscale[768:1024] = 2.0
    gscale[1792:2048] = 2.0
    shared = {
        "wihT": np.ascontiguousarray(Wih.T[:, perm] * gscale[None, :]).astype(BF),
        "whhT": np.ascontiguousarray(Whh.T[:, perm] * gscale[None, :]).astype(BF),
        "wxsT": np.ascontiguousarray(Wx_s.T).astype(BF),
        "whsT": np.ascontiguousarray(Wh_s.T).astype(BF),
        "wgT": np.ascontiguousarray(Wg.T).astype(BF),
        "wsT": np.ascontiguousarray(Ws.T).astype(BF),
        "wvT": np.ascontiguousarray(Wv.T).astype(BF),
        "wmlpT": np.ascontiguousarray(Wmlp.T).astype(BF),
        "b4": np.ascontiguousarray(((bih + bhh)[perm] * gscale).reshape(NMT, 128).T),
        "wh": np.ascontiguousarray(Wh_att[0]).astype(BF),
        "bmlp": np.ascontiguousarray(bmlp).astype(BF),
    }
    in_maps = []
    for c in range(NCORES):
        xi = x[c * BL : (c + 1) * BL]          # [BL, T, INP]
        Vi = V[c * BL : (c + 1) * BL]          # [BL, L, HID]
        xT = np.ascontiguousarray(xi.transpose(2, 1, 0).reshape(INP, NBT)).astype(BF)
        vflat = Vi.reshape(BL * L, HID)
        in_maps.append(dict(shared,
                            xT=xT,
                            vT=np.ascontiguousarray(vflat.T).astype(BF),
                            vnat=np.ascontiguousarray(vflat).astype(BF)))

    trace = os.environ.get("BASS_KERNEL_TRACE", "0") == "1"
    res = run_bass_kernel_spmd(nc, in_maps, core_ids=list(range(NCORES)), trace=trace)
    LAST_RESULT = res

    out = np.empty((B, T, VOCAB), np.float32)
    for c in range(NCORES):
        yc = np.asarray(res.results[c]["y"], dtype=np.float32).reshape(T, BL, VOCAB)
        out[c * BL : (c + 1) * BL] = yc.transpose(1, 0, 2)
    return out



# revision 57
# speedup vs baseline: 1.0410x; 1.0410x over previous
"""Adaptive-attention LSTM (B=32,T=64,HID=512,K=L=49,VOCAB=10000) on 8 TRN2 cores.

Strategy
--------
Data-parallel over batch: each core gets B_local=4 sequences. Everything is
computed in a "transposed" layout [feature-on-partition, (t,b)-on-free] so the
serial LSTM cell needs no transposes:

  phase A (batched): x_gates_T = Wih@x + (bih+bhh), x_sent_T = Wx_s@x,
                     Vk = V@Wv.T (replicated to [128, L*K] via a DRAM bounce)
  phase B (serial, 64 steps): only the LSTM cell. gates_T[2048, 4] accumulated
                     in PSUM from 64 bf16 LDW+matmuls (Whh.T stationary);
                     sigma/tanh on ACT, c/h updates on DVE. h_t, tanh(c_t)
                     stored for phase C.
  phase C (batched): spatial attention z/alpha, visual sentinel s/beta,
                     c_hat folded as  u = (e_how*s + e_z@V)/D + h  with
                     D = sum(e_z) + e_how  (exact softmax algebra; safe without
                     max-subtraction since |z| < 1), then the dominant GEMM
                     y = u @ Wmlp.T + bmlp with Wmlp streamed from HBM.

Host side only shards/transposes/casts inputs (bf16 for matmul operands) and
reassembles the output.
"""

import os
import sys
import types

for _p in ("/opt/pypackages", "/opt/trn_rl_repo"):
    if _p not in sys.path and os.path.isdir(_p):
        sys.path.insert(0, _p)


def _install_ntff_shim():
    """antenv.axon_hooks is missing in the trimmed repo; provide it so
    run_bass_kernel_spmd(trace=True) can reach the NTFF profile hook."""
    if "antenv.axon_hooks" in sys.modules:
        return
    try:
        from trn_agent_boot.trn_boot import _ntff_profile_via_ctypes

        hook = _ntff_profile_via_ctypes("/opt/axon/libaxon_pjrt.so")
    except Exception:
        hook = None
    m = types.ModuleType("antenv.axon_hooks")
    m.get_axon_ntff_profile_hook = lambda: hook
    m.set_axon_ntff_profile_hook = lambda h: None
    sys.modules["antenv.axon_hooks"] = m


_install_ntff_shim()


def _enable_ldw_opt():
    """compile flags hardcode --enable-ldw-opt=false; flip it so LDWEIGHTS can
    use the fast-weight-load path (bf16 weight tiles load 2x faster)."""
    import concourse.bass_utils as bu

    if os.environ.get("BASS_LDW_OPT", "1") != "1":
        return
    if getattr(bu, "_ldw_patched", False):
        return
    orig = bu.run_command

    def patched(argv, **kw):
        argv = [a.replace("--enable-ldw-opt=false", "--enable-ldw-opt=true")
                if isinstance(a, str) else a for a in argv]
        return orig(argv, **kw)

    bu.run_command = patched
    bu._ldw_patched = True


_enable_ldw_opt()

import ml_dtypes
import numpy as np

import concourse.bass as bass
import concourse.mybir as mybir
import concourse.tile as tile
from concourse.bass_utils import run_bass_kernel_spmd
from concourse.tile import add_dep_helper
from concourse.vector_clock import ScopedClock


def _patch_tile_drain():
    """This walrus build allows a single sync-wait per CTRL instruction; Tile's
    tail drain attaches one wait per live semaphore. Spread them over a chain
    of SP nops (sequential on SP -> identical semantics)."""
    if getattr(tile.TileContext, "_drain_patched", False):
        return

    def _drain_and_barrier(self, tick_clock, wait_clock):
        nc = self.nc
        probe = nc.sync.nop(nofuse=True)
        wait_clock.add_sem_waits(
            probe.ins, ScopedClock({None: tick_clock.global_clock})
        )
        waits = list(probe.ins.sync_info.on_wait)
        if len(waits) > 1:
            probe.ins.sync_info.on_wait = waits[:1]
            for i in range(1, len(waits)):
                extra = nc.sync.nop(nofuse=True)
                if extra.ins.sync_info is None:
                    extra.ins.sync_info = mybir.SyncInfo(
                        on_wait=waits[i : i + 1], on_update=[]
                    )
                else:
                    extra.ins.sync_info.on_wait = waits[i : i + 1]
        nc.sync.drain()
        nc.all_engine_barrier()
        assert self.sems is not None
        popped = nc._tile_sem_poison_stack.pop()
        assert popped is self._sem_poison
        nc.clear_and_free_semaphores(list(self.sems.allocated().values()))
        nc.all_engine_barrier()

    tile.TileContext._drain_and_barrier = _drain_and_barrier
    tile.TileContext._drain_patched = True

    # General pass: the ISA here allows a single sync-wait per instruction.
    # Before lowering, split any instruction with N>1 waits into N-1 preceding
    # single-wait NOPs on the same engine (engine streams execute in order, so
    # semantics are identical).
    _orig_lower = tile.TileContext._lower_ordered_insts

    def _split_multi_waits(self, ordered):
        nc = self.nc
        # Sweep 1: fuse standalone InstLdweights back into their InstMatmult
        # (self-loading form, ldweights=True). The standalone form defeats the
        # codegen fast-weight-load path (LDWEIGHTS measured at the slow
        # 128col/1.2GHz rate). Pair FIFO by PE stream order; merge sync lists.
        fuse = os.environ.get("BASS_FUSE_LDW", "1") == "1"
        for insts in ordered.values():
            if not fuse:
                break
            pending = []
            fused_out = []
            for inst in insts:
                tn = type(inst).__name__
                if tn == "InstLdweights":
                    pending.append(inst)
                    continue
                if tn == "InstMatmult" and getattr(inst, "ldweights", None) is not True \
                        and pending:
                    want = str(inst.ins[1])
                    idx = next((j for j, l in enumerate(pending)
                                if str(l.ins[0]) == want), 0)
                    ldw = pending.pop(idx)
                    inst.ldweights = True
                    lsi = getattr(ldw, "sync_info", None)
                    if lsi is not None and (lsi.on_wait or lsi.on_update):
                        si = inst.sync_info
                        if si is None:
                            inst.sync_info = mybir.SyncInfo(
                                on_wait=list(lsi.on_wait or []),
                                on_update=list(lsi.on_update or []),
                            )
                        else:
                            si.on_wait = list(lsi.on_wait or []) + list(si.on_wait or [])
                            si.on_update = list(si.on_update or []) + list(
                                lsi.on_update or []
                            )
                fused_out.append(inst)
            assert not pending, f"{len(pending)} unpaired Ldweights"
            insts[:] = fused_out

        # Sweep 2: this ISA allows one sync-wait per instruction; spill extras
        # onto preceding same-engine NOPs.
        for insts in ordered.values():
            out = []
            for inst in insts:
                si = getattr(inst, "sync_info", None)
                eng = getattr(inst, "engine", None)
                if si is not None and eng is not None and si.on_wait is not None \
                        and len(si.on_wait) > 1:
                    waits = list(si.on_wait)
                    for w in waits[:-1]:
                        out.append(mybir.InstNoOp(
                            name=nc.get_next_instruction_name(),
                            engine=eng,
                            bass_nofuse=True,
                            sync_info=mybir.SyncInfo(on_wait=[w], on_update=[]),
                        ))
                    si.on_wait = waits[-1:]
                out.append(inst)
            insts[:] = out
        return _orig_lower(self, ordered)

    tile.TileContext._lower_ordered_insts = _split_multi_waits


_patch_tile_drain()

F32 = mybir.dt.float32
BF16 = mybir.dt.bfloat16
FP8 = mybir.dt.float8e4
BF = ml_dtypes.bfloat16
F8 = ml_dtypes.float8_e4m3fn
AF = mybir.ActivationFunctionType
ALU = mybir.AluOpType

HID = 512
INP = 512
K = 49
L = 49
VOCAB = 10000
B, T = 32, 64
NCORES = 8
BL = B // NCORES          # 4 sequences per core
NBT = BL * T              # 256 (t-major: col = t*BL + b)
G4 = 4 * HID              # 2048
NKC = HID // 128          # 4 k-chunks
NMT = G4 // 128           # 16 gate m-tiles
NCH = 20                  # vocab chunks
CHN = VOCAB // NCH        # 500

LAST_RESULT = None        # BassKernelResults of the most recent run (for test.py)


def _bcast_ap(ap2d, count, pos=1):
    """Insert a zero-stride dim of `count` at free position `pos` of a 2-D AP."""
    dims = list(ap2d.ap)
    dims.insert(pos, [0, count])
    return bass.AP(tensor=ap2d.tensor, offset=ap2d.offset, ap=dims)


def _part_bcast(dram_ap, parts):
    """DRAM AP replicated over `parts` partitions (zero-stride partition dim)."""
    return bass.AP(
        tensor=dram_ap.tensor, offset=dram_ap.offset,
        ap=[[0, parts]] + list(dram_ap.ap),
    )


def _build():
    nc = bass.Bass()

    d_xT = nc.dram_tensor("xT", [INP, NBT], BF16, kind="ExternalInput")
    d_vT = nc.dram_tensor("vT", [HID, BL * L], BF16, kind="ExternalInput")
    d_vnat = nc.dram_tensor("vnat", [BL * L, HID], BF16, kind="ExternalInput")
    d_wihT = nc.dram_tensor("wihT", [INP, G4], BF16, kind="ExternalInput")
    d_whhT = nc.dram_tensor("whhT", [HID, G4], BF16, kind="ExternalInput")
    d_wxsT = nc.dram_tensor("wxsT", [INP, HID], BF16, kind="ExternalInput")
    d_whsT = nc.dram_tensor("whsT", [HID, HID], BF16, kind="ExternalInput")
    d_wgT = nc.dram_tensor("wgT", [HID, K], BF16, kind="ExternalInput")
    d_wsT = nc.dram_tensor("wsT", [HID, K], BF16, kind="ExternalInput")
    d_wvT = nc.dram_tensor("wvT", [HID, K], BF16, kind="ExternalInput")
    d_wmlpT = nc.dram_tensor("wmlpT", [HID, VOCAB], BF16, kind="ExternalInput")
    d_b4 = nc.dram_tensor("b4", [128, NMT], F32, kind="ExternalInput")
    d_wh = nc.dram_tensor("wh", [K], BF16, kind="ExternalInput")
    d_bmlp = nc.dram_tensor("bmlp", [VOCAB], BF16, kind="ExternalInput")
    d_y = nc.dram_tensor("y", [NBT, VOCAB], BF16, kind="ExternalOutput")

    ident = nc.inline_tensor(np.eye(128, dtype=np.float32), name="ident128")
    ident_bf = nc.inline_tensor(
        np.eye(128, dtype=np.float32).astype(ml_dtypes.bfloat16), name="ident128bf"
    )

    from contextlib import ExitStack

    with tile.TileContext(nc) as tc, ExitStack() as es:
        consts = es.enter_context(tc.tile_pool(name="consts", bufs=1))
        state = es.enter_context(tc.tile_pool(name="state", bufs=1))
        dram = es.enter_context(tc.tile_pool(name="dram", bufs=1, space="DRAM"))

        # ---- constant / weight loads ------------------------------------
        def load_kc(drm, cols, name, dt=BF16, eng=None):
            # single strided DMA: DRAM [(kc p), cols] -> SBUF [p, kc, cols]
            # (one queue issue instead of NKC; the issue cost ~650ns dominates)
            t = consts.tile([128, NKC, cols], dt, tag=name)
            src = drm[:]
            (eng or nc.sync).dma_start(
                out=t[:],
                in_=bass.AP(tensor=src.tensor, offset=src.offset,
                            ap=[[cols, 128], [128 * cols, NKC], [1, cols]]),
            )
            return t

        # Queue split: the ACT queue (scalar) gets only small/late tensors and
        # issues them FIRST -- its DMA issues block the ACT instruction stream,
        # so nothing big may sit in front of the phase-A psum->xg copies. The
        # big x-path weights stream on the SP queue in dependency order.
        b4 = consts.tile([128, NMT], F32, tag="b4")
        nc.scalar.dma_start(out=b4[:], in_=d_b4[:])
        vT = load_kc(d_vT, BL * L, "vT", eng=nc.scalar)
        wvT = load_kc(d_wvT, K, "wvT", eng=nc.scalar)
        wgT = load_kc(d_wgT, K, "wgT", eng=nc.scalar)
        wsT = load_kc(d_wsT, K, "wsT", eng=nc.scalar)
        whsT = load_kc(d_whsT, HID, "whsT", eng=nc.scalar)
        vnat = consts.tile([L, BL, HID], BF16, tag="vnat")
        vsrc = d_vnat[:]
        nc.scalar.dma_start(
            out=vnat[:],
            in_=bass.AP(tensor=vsrc.tensor, offset=vsrc.offset,
                        ap=[[HID, L], [L * HID, BL], [1, HID]]),
        )

        xT = load_kc(d_xT, NBT, "xT")
        wihT = load_kc(d_wihT, G4, "wihT")
        wxsT = load_kc(d_wxsT, HID, "wxsT")
        whhT = load_kc(d_whhT, G4, "whhT")

        ones_2b = consts.tile([128, 2, BL], F32, tag="ones_2b")
        nc.vector.memset(ones_2b[:], 1.0)
        wh_rep = consts.tile([128, K], BF16, tag="wh_rep")
        nc.scalar.dma_start(out=wh_rep[:], in_=_part_bcast(d_wh[:], 128))
        bmlp_rep = consts.tile([128, VOCAB], BF16, tag="bmlp_rep")
        # 2.5MB broadcast only needed by the MLP tail: keep it on the idle
        # SWDGE queue so it blocks neither HWDGE queue
        nc.gpsimd.dma_start(out=bmlp_rep[:], in_=_part_bcast(d_bmlp[:], 128))
        id_sb = consts.tile([128, 128], F32, tag="ident")
        nc.scalar.dma_start(out=id_sb[:], in_=ident[:])
        id_bf = consts.tile([128, 128], BF16, tag="ident_bf")
        nc.scalar.dma_start(out=id_bf[:], in_=ident_bf[:])
        ones1 = consts.tile([1, 128], F32, tag="ones1")
        nc.vector.memset(ones1[:], 1.0)

        # ---- persistent state -------------------------------------------
        xg = state.tile([128, T, NMT, BL], BF16, tag="xg")
        xs = state.tile([128, NKC, T, BL], F32, tag="xs")
        tanhc_bf = state.tile([128, NKC, T, BL], BF16, tag="tanhc_bf")
        h_bf = state.tile([128, NKC, T + 1, BL], BF16, tag="h_bf")
        c_st = state.tile([128, NKC, BL], F32, tag="c_st")
        vk_rep = state.tile([128, L, K], BF16, tag="vk_rep")
        s_bf = state.tile([128, NKC, NBT], BF16, tag="s_bf")
        u_bf = state.tile([128, NKC, NBT], BF16, tag="u_bf")
        hg_sb = state.tile([128, 2, K], BF16, tag="hg_sb")
        ez_sb = state.tile([128, 2, K], F32, tag="ez_sb")
        zt_sb = state.tile([128, 2, K], F32, tag="zt_sb")
        sws_sb = state.tile([128, 2, K], F32, tag="sws_sb")
        ezs_f = state.tile([128, 2, K], F32, tag="ezs_f")
        ezT = state.tile([L, NBT], BF16, tag="ezT")
        scal = state.tile([128, 2, 8], F32, tag="scal")
        frow = state.tile([1, NBT], F32, tag="frow")
        fbc = state.tile([128, NBT], F32, tag="fbc")

        d_vk = dram.tile([BL * L, K], BF16, tag="d_vk")
        d_vkrep = dram.tile([128, L * K], BF16, tag="d_vkrep")

        wmlp_sb = state.tile([128, NKC, VOCAB], BF16, tag="wmlp_sb")

        nc.vector.memset(c_st[:], 0.0)
        nc.vector.memset(h_bf[:, :, 0, :], 0.0)

        # ================= phase A: batched input projections =============
        with tc.tile_pool(name="pa_psum", bufs=4, space="PSUM") as pa_psum:
            for mt in range(NMT):
                p = pa_psum.tile([128, NBT], F32, tag="pa")
                for kc in range(NKC):
                    nc.tensor.matmul(
                        p[:],
                        lhsT=wihT[:, kc, mt * 128 : (mt + 1) * 128],
                        rhs=xT[:, kc, :],
                        start=(kc == 0),
                        stop=(kc == NKC - 1),
                    )
                nc.scalar.activation(
                    out=xg[:, :, mt, :],
                    in_=p[:].rearrange("p (t b) -> p t b", b=BL),
                    func=AF.Identity,
                    bias=b4[:, mt : mt + 1],
                    scale=1.0,
                )

        with tc.tile_pool(name="pa2_psum", bufs=4, space="PSUM") as pa_psum:
            pss = [pa_psum.tile([128, NBT], F32, tag="pa", name=f"ps{st}")
                   for st in range(NKC)]
            prev_mm = None
            for kc in range(NKC):
                for st in range(NKC):
                    mm = nc.tensor.matmul(
                        pss[st][:],
                        lhsT=wxsT[:, kc, st * 128 : (st + 1) * 128],
                        rhs=xT[:, kc, :],
                        start=(kc == 0),
                        stop=(kc == NKC - 1),
                        skip_group_check=True,
                    )
                    if prev_mm is not None:
                        add_dep_helper(mm.ins, prev_mm.ins, sync=False,
                                       reason="psum group order")
                    prev_mm = mm
            for st in range(NKC):
                nc.scalar.copy(
                    out=xs[:, st, :, :],
                    in_=pss[st][:].rearrange("p (t b) -> p t b", b=BL),
                )

            # Vk = V @ Wv.T  ->  DRAM bounce  ->  [128, L*K] partition replica
            for half in range(2):
                rows = 128 if half == 0 else BL * L - 128
                p = pa_psum.tile([128, K], F32, tag="pvk")
                for kc in range(NKC):
                    nc.tensor.matmul(
                        p[:rows, :],
                        lhsT=vT[:, kc, half * 128 : half * 128 + rows],
                        rhs=wvT[:, kc, :],
                        start=(kc == 0),
                        stop=(kc == NKC - 1),
                    )
                tmp = state.tile([128, K], BF16, tag=f"vkh{half}")
                nc.scalar.copy(out=tmp[:rows, :], in_=p[:rows, :])
                nc.sync.dma_start(
                    out=d_vk[half * 128 : half * 128 + rows, :], in_=tmp[:rows, :]
                )
            src = d_vk[:]
            dst = d_vkrep[:]
            nc.sync.dma_start(
                out=bass.AP(tensor=dst.tensor, offset=dst.offset,
                            ap=[[L * K * BL, 128 // BL], [L * K, BL], [1, L * K]]),
                in_=bass.AP(tensor=src.tensor, offset=src.offset,
                            ap=[[0, 128 // BL], [L * K, BL], [1, L * K]]),
            )
            nc.sync.dma_start(
                out=vk_rep[:].rearrange("p l k -> p (l k)"), in_=d_vkrep[:]
            )


        # ================= phase B: serial LSTM recurrence ================
        # Layout: gates permuted chunk-major on host: 2 chunks of hidden dims
        # (0:256, 256:512); within a chunk the 8 m-tiles are [i,i,f,f,o,o,g,g]
        # and the g rows are pre-scaled by 2 so tanh(g) = 2*sigmoid(2g)-1 and
        # ONE sigmoid covers all 32 psum cols. x_gates is DMA-preloaded into
        # PSUM (matmuls accumulate with start=False), so the chain per chunk is
        # sigmoid -> [stt tanh_g, cf, ig, c+] -> tanh_c -> h, with chunk 0's
        # chain overlapping chunk 1's matmuls and the next step's matmuls.
        with tc.tile_pool(name="pb_psum", bufs=4, space="PSUM") as pb_psum, \
             tc.tile_pool(name="pb_tmp", bufs=4) as pb_tmp:
            # stream the full Wmlp into SBUF on the otherwise-idle DMA queue
            # while the recurrence runs; the MLP phase then never touches HBM
            wsrc = d_wmlpT[:]
            nc.sync.dma_start(
                out=wmlp_sb[:],
                in_=bass.AP(tensor=wsrc.tensor, offset=wsrc.offset,
                            ap=[[VOCAB, 128], [128 * VOCAB, NKC], [1, VOCAB]]),
            )
            for t in range(T):
                pcs = []
                prev_mm = None
                for c in range(2):
                    pc = pb_psum.tile([128, 8, BL], F32, tag="pg")
                    # seed psum with x_gates via an identity matmul on the
                    # half-idle PE (keeps the preload off the busy ACT queue)
                    mm = nc.tensor.matmul(
                        pc[:].rearrange("p a b -> p (a b)"),
                        lhsT=id_bf[:],
                        rhs=xg[:, t, c * 8 : (c + 1) * 8, :].rearrange(
                            "p a b -> p (a b)"),
                        start=True, stop=False, skip_group_check=True,
                    )
                    if prev_mm is not None:
                        add_dep_helper(mm.ins, prev_mm.ins, sync=False,
                                       reason="psum group order")
                    prev_mm = mm
                    pcs.append(pc)
                for c in range(2):
                    for ic in range(2):
                        for ml in range(8):
                            mt = c * 8 + ml
                            for kk in range(2):
                                kc = ic * 2 + kk
                                mm = nc.tensor.matmul(
                                    pcs[c][:, ml, :],
                                    lhsT=whhT[:, kc, mt * 128 : (mt + 1) * 128],
                                    rhs=h_bf[:, kc, t, :],
                                    start=False,
                                    stop=(ic == 1 and kk == 1),
                                    skip_group_check=True,
                                )
                                if prev_mm is not None:
                                    add_dep_helper(mm.ins, prev_mm.ins, sync=False,
                                                   reason="psum group order")
                                prev_mm = mm

                acts = []
                for c in range(2):
                    a = pb_tmp.tile([128, 8, BL], F32, tag="act")
                    nc.scalar.activation(out=a[:], in_=pcs[c][:], func=AF.Sigmoid)
                    acts.append(a)
                for c in range(2):
                    a = acts[c]
                    cs = c_st[:, 2 * c : 2 * c + 2, :]
                    nc.vector.scalar_tensor_tensor(
                        out=a[:, 6:8, :], in0=a[:, 6:8, :], scalar=2.0,
                        in1=ones_2b[:], op0=ALU.mult, op1=ALU.subtract,
                    )
                    nc.vector.tensor_mul(cs, a[:, 2:4, :], cs)
                    ig = pb_tmp.tile([128, 2, BL], F32, tag="ig")
                    nc.vector.tensor_mul(ig[:], a[:, 0:2, :], a[:, 6:8, :])
                    nc.vector.tensor_add(cs, cs, ig[:])
                    nc.scalar.activation(
                        out=tanhc_bf[:, 2 * c : 2 * c + 2, t, :], in_=cs, func=AF.Tanh
                    )
                for c in range(2):
                    nc.vector.tensor_mul(
                        h_bf[:, 2 * c : 2 * c + 2, t + 1, :],
                        acts[c][:, 4:6, :],
                        tanhc_bf[:, 2 * c : 2 * c + 2, t, :],
                    )

        # ================= phase C: attention + sentinel ==================
        with tc.tile_pool(name="pc_psum", bufs=3, space="PSUM") as pc_psum, \
             tc.tile_pool(name="pc_cat", bufs=4, space="PSUM") as pc_cat, \
             tc.tile_pool(name="pc_tmp", bufs=2) as pc_tmp:
            # pin ACT order within phase C so the scheduler cannot interleave
            # EXP ops (different function table) with tanh/sigmoid ones: each
            # interleaving costs a 1.3us ACT_TABLE_LOAD
            _act_prev = [None]

            def _ach(op):
                if _act_prev[0] is not None:
                    add_dep_helper(op.ins, _act_prev[0].ins, sync=False,
                                   reason="act table order")
                _act_prev[0] = op
                return op

            for ti in range(2):
                cols = slice(1 + ti * 32, 1 + ti * 32 + 32)   # current h
                pcols = slice(0 + ti * 32, 0 + ti * 32 + 32)  # h_prev (shifted)
                bt = slice(ti * 128, (ti + 1) * 128)

                ph = pc_psum.tile([128, K], F32, tag="pcp")
                for kc in range(NKC):
                    nc.tensor.matmul(
                        ph[:], lhsT=h_bf[:, kc, cols, :], rhs=wgT[:, kc, :],
                        start=(kc == 0), stop=(kc == NKC - 1),
                    )
                _ach(nc.scalar.copy(out=hg_sb[:, ti, :], in_=ph[:]))

                # z-chain in half-L slices: DVE (add/mul/reduce) and ACT (tanh)
                # ping-pong across halves instead of serializing one big chain
                zin = pc_tmp.tile([128, L, K], BF16, tag="zin")
                for l0, l1 in ((0, 25), (25, L)):
                    nc.vector.tensor_add(
                        zin[:, l0:l1, :], vk_rep[:, l0:l1, :],
                        _bcast_ap(hg_sb[:, ti, :], l1 - l0),
                    )
                    _ach(nc.scalar.activation(
                        out=zin[:, l0:l1, :], in_=zin[:, l0:l1, :], func=AF.Tanh
                    ))
                    nc.vector.tensor_mul(
                        zin[:, l0:l1, :], zin[:, l0:l1, :],
                        _bcast_ap(wh_rep[:], l1 - l0),
                    )
                    nc.vector.tensor_reduce(
                        out=zt_sb[:, ti, l0:l1], in_=zin[:, l0:l1, :],
                        axis=mybir.AxisListType.X, op=ALU.add,
                    )

                for st in range(NKC):
                    ps = pc_psum.tile([128, 128], F32, tag="pcp")
                    for kc in range(NKC):
                        nc.tensor.matmul(
                            ps[:], lhsT=whsT[:, kc, st * 128 : (st + 1) * 128],
                            rhs=h_bf[:, kc, pcols, :],
                            start=(kc == 0), stop=(kc == NKC - 1),
                        )
                    pssb = pc_tmp.tile([128, 128], F32, tag="pssb")
                    nc.vector.tensor_add(
                        pssb[:].rearrange("p (t b) -> p t b", b=BL),
                        ps[:].rearrange("p (t b) -> p t b", b=BL),
                        xs[:, st, ti * 32 : (ti + 1) * 32, :],
                    )
                    _ach(nc.scalar.activation(out=pssb[:], in_=pssb[:], func=AF.Sigmoid))
                    nc.vector.tensor_mul(
                        s_bf[:, st, bt].rearrange("p (t b) -> p t b", b=BL),
                        pssb[:].rearrange("p (t b) -> p t b", b=BL),
                        tanhc_bf[:, st, ti * 32 : (ti + 1) * 32, :],
                    )

                pw = pc_psum.tile([128, K], F32, tag="pcp")
                for kc in range(NKC):
                    nc.tensor.matmul(
                        pw[:], lhsT=s_bf[:, kc, bt], rhs=wsT[:, kc, :],
                        start=(kc == 0), stop=(kc == NKC - 1),
                    )
                sws = sws_sb[:, ti, :]
                nc.vector.tensor_add(sws, pw[:], hg_sb[:, ti, :])
                _ach(nc.scalar.activation(out=sws, in_=sws, func=AF.Tanh))
                nc.vector.tensor_mul(sws, sws, wh_rep[:])
                nc.vector.tensor_reduce(
                    out=scal[:, ti, 1:2], in_=sws,
                    axis=mybir.AxisListType.X, op=ALU.add,
                )

            # pass 2: all exp/softmax work together so the ACT function table
            # switches to EXP exactly once instead of per-ti interleaving
            for ti in range(2):
                bt = slice(ti * 128, (ti + 1) * 128)
                _ach(nc.scalar.activation(out=ez_sb[:, ti, :], in_=zt_sb[:, ti, :], func=AF.Exp))
                _ach(nc.scalar.activation(out=scal[:, ti, 1:2], in_=scal[:, ti, 1:2], func=AF.Exp))
                nc.vector.tensor_reduce(
                    out=scal[:, ti, 0:1], in_=ez_sb[:, ti, :],
                    axis=mybir.AxisListType.X, op=ALU.add,
                )
                nc.vector.tensor_add(scal[:, ti, 2:3], scal[:, ti, 0:1], scal[:, ti, 1:2])
                nc.vector.reciprocal(scal[:, ti, 3:4], scal[:, ti, 2:3])
                nc.vector.tensor_mul(scal[:, ti, 4:5], scal[:, ti, 1:2], scal[:, ti, 3:4])
                nc.vector.tensor_scalar_mul(ezs_f[:, ti, :], ez_sb[:, ti, :], scal[:, ti, 3:4])
                pt = pc_psum.tile([K, 128], F32, tag="pcp")
                nc.tensor.transpose(pt[:], ezs_f[:, ti, :], id_sb[:])
                nc.vector.tensor_copy(ezT[:, bt], pt[:])
                pf = pc_psum.tile([1, 128], F32, tag="pcp")
                nc.tensor.transpose(pf[:], scal[:, ti, 4:5], id_sb[:])
                nc.vector.tensor_copy(frow[:, bt], pf[:])

            pfb = pc_psum.tile([128, NBT], F32, tag="pcp")
            nc.tensor.matmul(pfb[:], lhsT=ones1[:], rhs=frow[:], start=True, stop=True)
            nc.vector.tensor_copy(fbc[:], pfb[:])

            catts = []
            for hc in range(NKC):
                pc = pc_cat.tile([128, NBT], F32, tag="catt")
                for b in range(BL):
                    nc.tensor.matmul(
                        pc[:].rearrange("p (t b) -> p t b", b=BL)[:, :, b],
                        lhsT=vnat[:, b, hc * 128 : (hc + 1) * 128],
                        rhs=ezT[:].rearrange("p (t b) -> p t b", b=BL)[:, :, b],
                        start=True, stop=True,
                    )
                catts.append(pc)

            for hc in range(NKC):
                us = pc_tmp.tile([128, NBT], F32, tag="us")
                nc.vector.tensor_mul(us[:], s_bf[:, hc, :], fbc[:])
                nc.vector.tensor_add(us[:], us[:], catts[hc][:])
                nc.vector.tensor_add(
                    us[:].rearrange("p (t b) -> p t b", b=BL),
                    us[:].rearrange("p (t b) -> p t b", b=BL),
                    h_bf[:, hc, 1 : T + 1, :],
                )
                nc.vector.tensor_copy(u_bf[:, hc, :], us[:])

        # ================= MLP: y = u @ Wmlp.T + bmlp =====================
        with tc.tile_pool(name="mlp_ps", bufs=6, space="PSUM") as mlp_ps, \
             tc.tile_pool(name="mlp_out", bufs=6) as mlp_out:
            ydst = d_y[:]
            for nch in range(NCH):
                ysb = mlp_out.tile([128, 2, CHN], BF16, tag="ysb")
                for ti in range(2):
                    py = mlp_ps.tile([128, CHN], F32, tag="py")
                    for kc in range(NKC):
                        nc.tensor.matmul(
                            py[:], lhsT=u_bf[:, kc, ti * 128 : (ti + 1) * 128],
                            rhs=wmlp_sb[:, kc, nch * CHN : (nch + 1) * CHN],
                            start=(kc == 0), stop=(kc == NKC - 1),
                        )
                    nc.vector.tensor_add(
                        ysb[:, ti, :], py[:], bmlp_rep[:, nch * CHN : (nch + 1) * CHN]
                    )
                # one coalesced store per vocab chunk on the idle ACT queue
                nc.scalar.dma_start(
                    out=bass.AP(tensor=ydst.tensor, offset=ydst.offset + nch * CHN,
                                ap=[[VOCAB, 128], [128 * VOCAB, 2], [1, CHN]]),
                    in_=ysb[:],
                )

    return nc


_NC_CACHE = None


def kernel(**inputs):
    global _NC_CACHE, LAST_RESULT
    x = np.asarray(inputs["x"], np.float32)
    V = np.asarray(inputs["V"], np.float32)
    Wih = np.asarray(inputs["Wih"], np.float32)
    Whh = np.asarray(inputs["Whh"], np.float32)
    bih = np.asarray(inputs["bih"], np.float32)
    bhh = np.asarray(inputs["bhh"], np.float32)
    Wx_s = np.asarray(inputs["Wx_s"], np.float32)
    Wh_s = np.asarray(inputs["Wh_s"], np.float32)
    Wv = np.asarray(inputs["Wv"], np.float32)
    Wg = np.asarray(inputs["Wg"], np.float32)
    Wh_att = np.asarray(inputs["Wh_att"], np.float32)
    Ws = np.asarray(inputs["Ws"], np.float32)
    Wmlp = np.asarray(inputs["Wmlp"], np.float32)
    bmlp = np.asarray(inputs["bmlp"], np.float32)

    if _NC_CACHE is None:
        _NC_CACHE = _build()
    nc = _NC_CACHE

    # permute gates chunk-major: chunk c (hidden dims c*256:(c+1)*256) holds
    # [i, f, o, g] blocks of 256; g rows are pre-scaled by 2 so the kernel can
    # compute tanh(g) = 2*sigmoid(2g) - 1 with a single fused sigmoid.
    perm = np.concatenate([
        np.concatenate([
            np.arange(gb * 512 + c * 256, gb * 512 + (c + 1) * 256)
            for gb in (0, 1, 3, 2)  # i, f, o, g
        ])
        for c in (0, 1)
    ])
    gscale = np.ones(4 * HID, np.float32)
    gscale[768:1024] = 2.0
    gscale[1792:2048] = 2.0
    shared = {
        "wihT": np.ascontiguousarray(Wih.T[:, perm] * gscale[None, :]).astype(BF),
        "whhT": np.ascontiguousarray(Whh.T[:, perm] * gscale[None, :]).astype(BF),
        "wxsT": np.ascontiguousarray(Wx_s.T).astype(BF),
        "whsT": np.ascontiguousarray(Wh_s.T).astype(BF),
        "wgT": np.ascontiguousarray(Wg.T).astype(BF),
        "wsT": np.ascontiguousarray(Ws.T).astype(BF),
        "wvT": np.ascontiguousarray(Wv.T).astype(BF),
        "wmlpT": np.ascontiguousarray(Wmlp.T).astype(BF),
        "b4": np.ascontiguousarray(((bih + bhh)[perm] * gscale).reshape(NMT, 128).T),
        "wh": np.ascontiguousarray(Wh_att[0]).astype(BF),
        "bmlp": np.ascontiguousarray(bmlp).astype(BF),
    }
    in_maps = []
    for c in range(NCORES):
        xi = x[c * BL : (c + 1) * BL]          # [BL, T, INP]
        Vi = V[c * BL : (c + 1) * BL]          # [BL, L, HID]
        xT = np.ascontiguousarray(xi.transpose(2, 1, 0).reshape(INP, NBT)).astype(BF)
        vflat = Vi.reshape(BL * L, HID)
        in_maps.append(dict(shared,
                            xT=xT,
                            vT=np.ascontiguousarray(vflat.T).astype(BF),
                            vnat=np.ascontiguousarray(vflat).astype(BF)))

    trace = os.environ.get("BASS_KERNEL_TRACE", "0") == "1"
    res = run_bass_kernel_spmd(nc, in_maps, core_ids=list(range(NCORES)), trace=trace)
    LAST_RESULT = res

    out = np.empty((B, T, VOCAB), np.float32)
    for c in range(NCORES):
        yc = np.asarray(res.results[c]["y"], dtype=np.float32).reshape(T, BL, VOCAB)
        out[c * BL : (c + 1) * BL] = yc.transpose(1, 0, 2)
    return out

